# revision 4
# baseline (speedup 1.0000x reference)
"""DenseCapsule dynamic-routing kernel for 8 Trainium2 NeuronCores.

Problem: x[B=32,I=2048,D=16], w_ij[J=64,I=2048,C=32,D=16]
  u_hat = einsum('bid,jicd->bjic', x, w_ij)
  5 routing iterations (softmax over J, s = sum_i c*u_hat, v = squash(s),
  b += sum_c v*u_hat), return v [B,J,C].

Sharding: input capsules I are split 8 ways (I_LOC=256 per core).  The
softmax over J is then core-local; the only collective is an AllReduce of
the per-core partial s [B,J,C] (128 KB fp16) once per iteration.

v2 group body (vs v0): the 5-op add-tree for the c-reduction is one DVE
reduce_sum (fp16 out); Z is a free accum_out on the ACT exp; 1/Z moves
from a slow DVE AP-scalar mul (ln tiles, ~1.3us each) into the ACT
broadcast's scale operand, making the PE stationary a constant d1.
"""

import numpy as np

B, I, D, J, C = 32, 2048, 16, 64, 32
NCORES = 8
I_LOC = I // NCORES      # 256
G = 4                    # i's per block (G*D = 64 contraction partitions)
NBLK = I_LOC // G        # 64
JC = J * C               # 2048
ITERS = 5
EPS = 1e-7
NCH = 4                  # 512-wide matmul chunks over JC
GR = 4                   # i-blocks per phase-2 tile group
NG = NBLK // GR          # 16 groups per iteration

_CACHE = {}


def _build(repeats=1):
    import concourse.bacc as bacc
    import concourse.mybir as mybir
    from concourse import tile

    f32 = mybir.dt.float32
    fp16 = mybir.dt.float16
    Act = mybir.ActivationFunctionType
    Alu = mybir.AluOpType
    X = mybir.AxisListType.X

    nc = bacc.Bacc("TRN2", target_bir_lowering=False, debug=False,
                   num_devices=NCORES)
    NP = NBLK // 2   # block pairs: two G*D=64 blocks stacked on 128 partitions
    xd = nc.dram_tensor("xd", [NP, 128, 128], fp16, kind="ExternalInput").ap()
    xs = nc.dram_tensor("xs", [NP, 128, B], fp16, kind="ExternalInput").ap()
    wm = nc.dram_tensor("wm", [NP, 128, JC], fp16, kind="ExternalInput").ap()
    d1 = nc.dram_tensor("d1", [128, B], fp16, kind="ExternalInput").ap()
    v_out = nc.dram_tensor("v_out", [B, JC], f32, kind="ExternalOutput").ap()

    with tile.TileContext(nc) as tc:
        with tc.tile_pool(name="const", bufs=1) as constp, \
             tc.tile_pool(name="io", bufs=2) as iop, \
             tc.tile_pool(name="u", bufs=1) as up, \
             tc.tile_pool(name="work", bufs=2) as wp, \
             tc.tile_pool(name="small", bufs=1) as sp, \
             tc.tile_pool(name="spg", bufs=3) as spg, \
             tc.tile_pool(name="psum", bufs=4, space="PSUM") as pp, \
             tc.tile_pool(name="spsum", bufs=1, space="PSUM") as spp, \
             tc.tile_pool(name="ud", bufs=1, space="DRAM") as udp, \
             tc.tile_pool(name="ar", bufs=2, space="DRAM") as arp:

            d1_t = constp.tile([128, B], fp16)
            nc.sync.dma_start(d1_t[:], d1[:])
            eps_t = constp.tile([B, 1], f32, tag="eps")
            nc.gpsimd.memset(eps_t[:], EPS)
            b_tiles = []                                 # routing logits
            for gi in range(NG):
                bt = constp.tile([128, GR * J], f32, tag=f"b{gi}")
                b_tiles.append(bt)
            u_store = udp.tile([NBLK, 128, JC], fp16)

            for _rep in range(repeats):
                for bt in b_tiles:
                    nc.gpsimd.memset(bt[:], 0.0)

                # ---- Phase 1: u_hat production + iteration-1 s accumulation
                # s^1 = sum_i u is computed straight from x,W (stationary xs =
                # stacked x block, no block-diagonal) so the s-matmuls depend
                # only on wm_t, never on the PSUM->SBUF copies: PE streams
                # LDW/MM back-to-back with zero DVE/ACT dependencies.
                s_ps = spp.tile([B, JC], f32, tag="s")
                for pr in range(NP):
                    xd_t = iop.tile([128, 128], fp16, tag="xd_t")
                    nc.sync.dma_start(xd_t[:], xd[pr])
                    xs_t = iop.tile([128, B], fp16, tag="xs_t")
                    nc.sync.dma_start(xs_t[:], xs[pr])
                    wm_t = iop.tile([128, JC], fp16, tag="wm_t", bufs=3)
                    nc.sync.dma_start(wm_t[:, :JC // 2], wm[pr][:, :JC // 2])
                    nc.sync.dma_start(wm_t[:, JC // 2:], wm[pr][:, JC // 2:])
                    u16a = iop.tile([128, JC], fp16, tag="u16a")
                    u16b = iop.tile([128, JC], fp16, tag="u16b")
                    for ch in range(NCH):
                        sl = slice(ch * 512, (ch + 1) * 512)
                        psa = pp.tile([128, 512], f32, tag="psa", bufs=2)
                        nc.tensor.matmul(psa[:], xd_t[0:64, :], wm_t[0:64, sl],
                                         start=True, stop=True,
                                         tile_position=(0, 0))
                        psb = pp.tile([128, 512], f32, tag="psb", bufs=2)
                        nc.tensor.matmul(psb[:], xd_t[64:128, :], wm_t[64:128, sl],
                                         start=True, stop=True,
                                         tile_position=(64, 0))
                        # PSUM evac 6:2 DVE:ACT — measured DVE cast 0.65us vs
                        # ACT copy 1.55us per [128,512] tile
                        if ch == 0:
                            nc.vector.tensor_copy(u16a[:, sl], psa[:])
                            nc.scalar.copy(u16b[:, sl], psb[:])
                        elif ch == 1:
                            nc.scalar.copy(u16a[:, sl], psa[:])
                            nc.vector.tensor_copy(u16b[:, sl], psb[:])
                        else:
                            nc.vector.tensor_copy(u16a[:, sl], psa[:])
                            nc.vector.tensor_copy(u16b[:, sl], psb[:])
                    for ch in range(NCH):
                        sl = slice(ch * 512, (ch + 1) * 512)
                        nc.tensor.matmul(s_ps[:, sl], xs_t[:], wm_t[:, sl],
                                         start=(pr == 0), stop=(pr == NP - 1))
                    nc.sync.dma_start(u_store[2 * pr], u16a[:])
                    nc.sync.dma_start(u_store[2 * pr + 1], u16b[:])

                # ---- Phase 2: routing iterations
                for it in range(1, ITERS + 1):
                    # v^{it} from the s accumulated for iteration `it`.
                    # AllReduce payload in fp16 (halves collective bytes; the
                    # 8-way partial sums tolerate fp16 rounding).
                    s_sb = sp.tile([B, JC], fp16, tag="s_sb")
                    nc.scalar.activation(s_sb[:], s_ps[:], Act.Copy, bias=0.0)
                    ar_in = arp.tile([B, JC], fp16, tag="ar_in")
                    ar_out = arp.tile([B, JC], fp16, tag="ar_out")
                    nc.sync.dma_start(ar_in[:], s_sb[:])
                    nc.gpsimd.collective_compute(
                        "AllReduce", Alu.add,
                        replica_groups=[list(range(NCORES))],
                        ins=[ar_in.opt()], outs=[ar_out.opt()],
                    )
                    s_full = sp.tile([B, JC], fp16, tag="s_full")
                    nc.sync.dma_start(s_full[:], ar_out[:])

                    # squash: v0 = s+eps; scale = sqrt(n)/(1+n), n = sum v0^2
                    # sq on ACT (fused +EPS bias) in parallel with v0 on DVE
                    v0 = sp.tile([B, JC], f32, tag="v0")
                    nc.vector.tensor_scalar_add(v0[:], s_full[:], EPS)
                    sq = sp.tile([B, JC], fp16, tag="s_sb")
                    nc.scalar.activation(sq[:], s_full[:], Act.Square,
                                         bias=eps_t[:])
                    norm = sp.tile([B, J], f32, tag="norm")
                    nc.vector.reduce_sum(norm[:],
                                         sq[:].rearrange("p (j c) -> p j c", c=C),
                                         axis=X)
                    rt = sp.tile([B, J], f32, tag="rt")
                    nc.scalar.activation(rt[:], norm[:], Act.Sqrt)
                    np1 = sp.tile([B, J], f32, tag="np1")
                    nc.vector.tensor_scalar_add(np1[:], norm[:], 1.0)
                    inv1 = sp.tile([B, J], f32, tag="inv1")
                    nc.vector.reciprocal(inv1[:], np1[:])
                    invd = sp.tile([B, J], f32, tag="invd")
                    nc.vector.tensor_mul(invd[:], rt[:], inv1[:])
                    if it == ITERS:
                        v_sb = v0    # in-place: v0's last use
                        nc.vector.tensor_mul(
                            v_sb[:].rearrange("p (j c) -> p j c", c=C),
                            v0[:].rearrange("p (j c) -> p j c", c=C),
                            invd[:].rearrange("p (j one) -> p j one", one=1)
                                   .broadcast_to((B, J, C)))
                        nc.sync.dma_start(v_out[:], v_sb[:])
                        break

                    # fuse the squash scale directly into the fp16 v used by
                    # the group loop (skips the separate f32 v_sb + copy)
                    v16 = sp.tile([B, JC], fp16, tag="s_sb")
                    nc.vector.tensor_mul(
                        v16[:].rearrange("p (j c) -> p j c", c=C),
                        v0[:].rearrange("p (j c) -> p j c", c=C),
                        invd[:].rearrange("p (j one) -> p j one", one=1)
                               .broadcast_to((B, J, C)))
                    v_rep = constp.tile([128, JC], fp16, tag="v_rep")
                    for g in range(G):
                        nc.sync.dma_start(v_rep[g * B:(g + 1) * B, :], v16[:])

                    s_ps = spp.tile([B, JC], f32, tag="s")

                    def stage_b(u_t, e_rep, g0):
                        # s += (e/Z) * u: e/Z applied on DVE; stationary is the
                        # constant block-diag delta d1 (1/Z folded into e_rep)
                        prod2 = wp.tile([128, GR * JC], fp16, tag="prod1", bufs=3)
                        nc.vector.tensor_mul(prod2[:], u_t[:], e_rep[:])
                        for n in range(GR):
                            blk = g0 + n
                            for ch in range(NCH):
                                sl = slice(n * JC + ch * 512,
                                           n * JC + (ch + 1) * 512)
                                nc.tensor.matmul(s_ps[:, ch * 512:(ch + 1) * 512],
                                                 d1_t[:], prod2[:, sl],
                                                 start=(blk == 0),
                                                 stop=(blk == NBLK - 1))

                    pending = None
                    for g0 in range(0, NBLK, GR):
                        ggr = GR * J                     # 256 j-slots per group
                        b_g = b_tiles[g0 // GR]
                        u_t = up.tile([128, GR * JC], fp16, tag="u_t", bufs=3)
                        nc.sync.dma_start(
                            u_t[:].rearrange("p (n f) -> p n f", n=GR),
                            u_store[g0:g0 + GR].rearrange("n p f -> p n f"))
                        # logits update t = sum_c u*v: one DVE mul + one DVE
                        # segmented reduce (fp16 accumulate; logits tolerate
                        # fp16 rounding, same as v0's pairwise fp16 tree).
                        prod1 = wp.tile([128, GR * JC], fp16, tag="prod1", bufs=3)
                        nc.vector.tensor_mul(
                            prod1[:].rearrange("p (n f) -> p n f", n=GR),
                            u_t[:].rearrange("p (n f) -> p n f", n=GR),
                            v_rep[:].rearrange("p (o f) -> p o f", o=1)
                                    .broadcast_to((128, GR, JC)))
                        # pairwise fp16 tree on DVE: every level keeps 2B
                        # dtypes + unit stride, so all levels run in 2x mode
                        # (a single segmented reduce_sum measures 1x: 8.2us).
                        p3 = prod1[:].rearrange("p (s c) -> p s c", s=ggr)
                        nc.vector.tensor_add(p3[:, :, 0:16], p3[:, :, 0:16],
                                             p3[:, :, 16:32])
                        nc.vector.tensor_add(p3[:, :, 0:8], p3[:, :, 0:8],
                                             p3[:, :, 8:16])
                        nc.vector.tensor_add(p3[:, :, 0:4], p3[:, :, 0:4],
                                             p3[:, :, 4:8])
                        nc.vector.tensor_add(p3[:, :, 0:2], p3[:, :, 0:2],
                                             p3[:, :, 2:4])
                        t16 = spg.tile([128, ggr], fp16, tag="t16", bufs=2)
                        nc.vector.tensor_add(
                            t16[:].rearrange("p (s c) -> p s c", s=ggr),
                            p3[:, :, 0:1], p3[:, :, 1:2])
                        # b += t stays on DVE: mixed f32+fp16 is 1x but tiny,
                        # and avoids a cross-engine hop (Pool adds ~4us/op)
                        nc.vector.tensor_add(b_g[:], b_g[:], t16[:])
                        # core-local softmax over j (all J present).  Z comes
                        # free from the exp's accum_out; 1/Z is applied by the
                        # ACT broadcast's per-partition scale, so the PE
                        # stationary stays the constant d1 delta.
                        nmx = spg.tile([128, GR], f32, tag="nmx")
                        nc.vector.reduce_max(
                            nmx[:], b_g[:].rearrange("p (n j) -> p n j", n=GR),
                            axis=X, negate=True)
                        e_grp = spg.tile([128, ggr], fp16, tag="e_grp")
                        zacc = spg.tile([128, GR], f32, tag="zacc")
                        for n in range(GR):
                            nc.scalar.activation(
                                e_grp[:, n * J:(n + 1) * J],
                                b_g[:, n * J:(n + 1) * J],
                                Act.Exp, bias=nmx[:, n:n + 1],
                                accum_out=zacc[:, n:n + 1])
                        invz = spg.tile([128, GR], f32, tag="invz")
                        nc.vector.reciprocal(invz[:], zacc[:])
                        e_rep = wp.tile([128, GR * JC], fp16, tag="e_rep", bufs=2)
                        for n in range(GR):
                            nc.scalar.activation(
                                e_rep[:, n * JC:(n + 1) * JC]
                                    .rearrange("p (j c) -> p j c", c=C),
                                e_grp[:, n * J:(n + 1) * J]
                                    .rearrange("p (j o) -> p j o", o=1)
                                    .broadcast_to((128, J, C)),
                                Act.Copy, bias=0.0, scale=invz[:, n:n + 1])
                        # software pipeline: emit the previous group's
                        # prod2+matmuls after this group's A-stage so DVE/PE
                        # have ready work while ACT builds this e_rep
                        if pending is not None:
                            stage_b(*pending)
                        pending = (u_t, e_rep, g0)
                    stage_b(*pending)

    nc.compile()
    return nc


def _prep_inputs(x, w_ij):
    """Host-side shard + layout. Returns per-core in_maps."""
    x_t = np.ascontiguousarray(x.transpose(1, 2, 0)).astype(np.float16)   # [I,D,B]
    w_t = np.ascontiguousarray(w_ij.transpose(1, 3, 0, 2)).astype(np.float16)  # [I,D,J,C]
    d1 = np.tile(np.eye(B, dtype=np.float16), (G, 1))                     # [128,B]
    in_maps = []
    for k in range(NCORES):
        xs = x_t[k * I_LOC:(k + 1) * I_LOC].reshape(NBLK, G, D, B)
        xd = np.zeros((NBLK, G * D, 128), np.float16)
        for g in range(G):
            xd[:, g * D:(g + 1) * D, g * B:(g + 1) * B] = xs[:, g]
        ws = w_t[k * I_LOC:(k + 1) * I_LOC].reshape(NBLK // 2, 2 * G * D, JC)
        # 1/J folded here so iteration 1's s needs no extra scale pass
        xsk = np.ascontiguousarray(
            x_t[k * I_LOC:(k + 1) * I_LOC].reshape(NBLK // 2, 2 * G * D, B)
            .astype(np.float32) / J).astype(np.float16)
        in_maps.append({"xd": xd.reshape(NBLK // 2, 2 * G * D, 128),
                        "xs": xsk, "wm": np.ascontiguousarray(ws),
                        "d1": d1})
    return in_maps


def kernel(x, w_ij, _trace=False):
    from concourse import bass_utils

    if "nc" not in _CACHE:
        _CACHE["nc"] = _build()
    nc = _CACHE["nc"]
    in_maps = _prep_inputs(np.asarray(x), np.asarray(w_ij))
    res = bass_utils.run_bass_kernel_spmd(
        nc, in_maps, core_ids=list(range(NCORES)), trace=_trace)
    _CACHE["last_result"] = res
    v = res.results[0]["v_out"].reshape(B, J, C).astype(np.float32)
    return v


# revision 6
# speedup vs baseline: 1.2209x; 1.2209x over previous
"""DenseCapsule dynamic-routing kernel for 8 Trainium2 NeuronCores.

Problem: x[B=32,I=2048,D=16], w_ij[J=64,I=2048,C=32,D=16]
  u_hat = einsum('bid,jicd->bjic', x, w_ij)
  5 routing iterations (softmax over J, s = sum_i c*u_hat, v = squash(s),
  b += sum_c v*u_hat), return v [B,J,C].

Sharding: input capsules I are split 8 ways (I_LOC=256 per core).  The
softmax over J is then core-local; the only collective is an AllReduce of
the per-core partial s [B,J,C] (128 KB fp16) once per iteration.

v3: the free axis of every big tile is (c, j) instead of (j, c).  With j
innermost, the per-(i,j) softmax weight e broadcasts along the MIDDLE
axis (stride-0) while the innermost stays unit-stride fp16 — so DVE's
prod2 reads e_grp directly through a broadcast view in 2x mode and the
[128,8192] e_rep materialization (4 ACT copies, 8us/group, the v0/v2
pipeline stall) disappears.  1/Z is folded into the PE stationary
(ln_n = d1 * invz_n) built on ACT where AP-scalar reads are cheap
(213ns vs 1285ns on DVE).  The c-reduction for the logit update is a
pairwise fp16 tree on DVE (all levels 2x; a single segmented reduce_sum
measures 1x).  Host transposes the [B,(C,J)] result back to [B,J,C].
"""

import numpy as np

B, I, D, J, C = 32, 2048, 16, 64, 32
NCORES = 8
I_LOC = I // NCORES      # 256
G = 4                    # i's per block (G*D = 64 contraction partitions)
NBLK = I_LOC // G        # 64
JC = J * C               # 2048 (free size; layout is (c, j) in-kernel)
ITERS = 5
EPS = 1e-7
NCH = 4                  # 512-wide matmul chunks over the (c,j) free axis
GR = 4                   # i-blocks per phase-2 tile group
NG = NBLK // GR          # 16 groups per iteration

_CACHE = {}


def _build(repeats=1):
    import concourse.bacc as bacc
    import concourse.mybir as mybir
    from concourse import tile

    f32 = mybir.dt.float32
    fp16 = mybir.dt.float16
    Act = mybir.ActivationFunctionType
    Alu = mybir.AluOpType
    X = mybir.AxisListType.X

    nc = bacc.Bacc("TRN2", target_bir_lowering=False, debug=False,
                   num_devices=NCORES)
    NP = NBLK // 2   # block pairs: two G*D=64 blocks stacked on 128 partitions
    xd = nc.dram_tensor("xd", [NP, 128, 128], fp16, kind="ExternalInput").ap()
    xs = nc.dram_tensor("xs", [NP, 128, B], fp16, kind="ExternalInput").ap()
    wm = nc.dram_tensor("wm", [NP, 128, JC], fp16, kind="ExternalInput").ap()
    d1 = nc.dram_tensor("d1", [128, B], fp16, kind="ExternalInput").ap()
    v_out = nc.dram_tensor("v_out", [B, JC], f32, kind="ExternalOutput").ap()

    with tile.TileContext(nc) as tc:
        with tc.tile_pool(name="const", bufs=1) as constp, \
             tc.tile_pool(name="io", bufs=2) as iop, \
             tc.tile_pool(name="u", bufs=1) as up, \
             tc.tile_pool(name="work", bufs=2) as wp, \
             tc.tile_pool(name="small", bufs=1) as sp, \
             tc.tile_pool(name="spg", bufs=3) as spg, \
             tc.tile_pool(name="psum", bufs=4, space="PSUM") as pp, \
             tc.tile_pool(name="spsum", bufs=1, space="PSUM") as spp, \
             tc.tile_pool(name="ud", bufs=1, space="DRAM") as udp, \
             tc.tile_pool(name="ar", bufs=2, space="DRAM") as arp:

            d1_t = constp.tile([128, B], fp16)
            nc.sync.dma_start(d1_t[:], d1[:])
            eps_t = constp.tile([B, 1], f32, tag="eps")
            nc.gpsimd.memset(eps_t[:], EPS)
            b_tiles = []                                 # routing logits
            for gi in range(NG):
                bt = constp.tile([128, GR * J], f32, tag=f"b{gi}")
                b_tiles.append(bt)
            u_store = udp.tile([NBLK, 128, JC], fp16)

            for _rep in range(repeats):
                for bt in b_tiles:
                    nc.gpsimd.memset(bt[:], 0.0)

                # ---- Phase 1: u_hat production + iteration-1 s accumulation
                # s^1 = sum_i u is computed straight from x,W (stationary xs =
                # stacked x block, no block-diagonal) so the s-matmuls depend
                # only on wm_t, never on the PSUM->SBUF copies: PE streams
                # LDW/MM back-to-back with zero DVE/ACT dependencies.
                s_ps = spp.tile([B, JC], f32, tag="s")
                for pr in range(NP):
                    xd_t = iop.tile([128, 128], fp16, tag="xd_t")
                    nc.sync.dma_start(xd_t[:], xd[pr])
                    xs_t = iop.tile([128, B], fp16, tag="xs_t")
                    nc.sync.dma_start(xs_t[:], xs[pr])
                    wm_t = iop.tile([128, JC], fp16, tag="wm_t", bufs=3)
                    nc.sync.dma_start(wm_t[:, :JC // 2], wm[pr][:, :JC // 2])
                    nc.sync.dma_start(wm_t[:, JC // 2:], wm[pr][:, JC // 2:])
                    u16a = iop.tile([128, JC], fp16, tag="u16a")
                    u16b = iop.tile([128, JC], fp16, tag="u16b")
                    for ch in range(NCH):
                        sl = slice(ch * 512, (ch + 1) * 512)
                        psa = pp.tile([128, 512], f32, tag="psa", bufs=2)
                        nc.tensor.matmul(psa[:], xd_t[0:64, :], wm_t[0:64, sl],
                                         start=True, stop=True,
                                         tile_position=(0, 0))
                        psb = pp.tile([128, 512], f32, tag="psb", bufs=2)
                        nc.tensor.matmul(psb[:], xd_t[64:128, :], wm_t[64:128, sl],
                                         start=True, stop=True,
                                         tile_position=(64, 0))
                        # PSUM evac 6:2 DVE:ACT — measured DVE cast 0.65us vs
                        # ACT copy 1.55us per [128,512] tile
                        if ch == 0:
                            nc.vector.tensor_copy(u16a[:, sl], psa[:])
                            nc.scalar.copy(u16b[:, sl], psb[:])
                        elif ch == 1:
                            nc.scalar.copy(u16a[:, sl], psa[:])
                            nc.vector.tensor_copy(u16b[:, sl], psb[:])
                        else:
                            nc.vector.tensor_copy(u16a[:, sl], psa[:])
                            nc.vector.tensor_copy(u16b[:, sl], psb[:])
                    for ch in range(NCH):
                        sl = slice(ch * 512, (ch + 1) * 512)
                        nc.tensor.matmul(s_ps[:, sl], xs_t[:], wm_t[:, sl],
                                         start=(pr == 0), stop=(pr == NP - 1))
                    nc.sync.dma_start(u_store[2 * pr], u16a[:])
                    nc.sync.dma_start(u_store[2 * pr + 1], u16b[:])

                # ---- Phase 2: routing iterations
                for it in range(1, ITERS + 1):
                    # v^{it} from the s accumulated for iteration `it`.
                    # AllReduce payload in fp16 (halves collective bytes; the
                    # 8-way partial sums tolerate fp16 rounding).
                    s_sb = sp.tile([B, JC], fp16, tag="s_sb")
                    nc.scalar.activation(s_sb[:], s_ps[:], Act.Copy, bias=0.0)
                    ar_in = arp.tile([B, JC], fp16, tag="ar_in")
                    ar_out = arp.tile([B, JC], fp16, tag="ar_out")
                    nc.sync.dma_start(ar_in[:], s_sb[:])
                    nc.gpsimd.collective_compute(
                        "AllReduce", Alu.add,
                        replica_groups=[list(range(NCORES))],
                        ins=[ar_in.opt()], outs=[ar_out.opt()],
                    )
                    s_full = sp.tile([B, JC], fp16, tag="s_full")
                    nc.sync.dma_start(s_full[:], ar_out[:])

                    # squash: v0 = s+eps; scale = sqrt(n)/(1+n), n = sum v0^2
                    # sq on ACT (fused +EPS bias) in parallel with v0 on DVE.
                    # norm = sum_c v0^2: c is the OUTER free axis now, so the
                    # reduction is a contiguous fp16 halving tree (2x mode).
                    v0 = sp.tile([B, JC], f32, tag="v0")
                    nc.vector.tensor_scalar_add(v0[:], s_full[:], EPS)
                    sq = sp.tile([B, JC], fp16, tag="s_sb")
                    nc.scalar.activation(sq[:], s_full[:], Act.Square,
                                         bias=eps_t[:])
                    h = JC // 2
                    while h >= J:
                        nc.vector.tensor_add(sq[:, 0:h], sq[:, 0:h],
                                             sq[:, h:2 * h])
                        h //= 2
                    norm = sp.tile([B, J], f32, tag="norm")
                    nc.vector.tensor_copy(norm[:], sq[:, 0:J])
                    rt = sp.tile([B, J], f32, tag="rt")
                    nc.scalar.activation(rt[:], norm[:], Act.Sqrt)
                    np1 = sp.tile([B, J], f32, tag="np1")
                    nc.vector.tensor_scalar_add(np1[:], norm[:], 1.0)
                    inv1 = sp.tile([B, J], f32, tag="inv1")
                    nc.vector.reciprocal(inv1[:], np1[:])
                    invd = sp.tile([B, J], f32, tag="invd")
                    nc.vector.tensor_mul(invd[:], rt[:], inv1[:])
                    if it == ITERS:
                        v_sb = v0    # in-place: v0's last use
                        nc.vector.tensor_mul(
                            v_sb[:].rearrange("p (c j) -> p c j", j=J),
                            v0[:].rearrange("p (c j) -> p c j", j=J),
                            invd[:].rearrange("p (one j) -> p one j", one=1)
                                   .broadcast_to((B, C, J)))
                        nc.sync.dma_start(v_out[:], v_sb[:])
                        break

                    # fuse the squash scale directly into the fp16 v used by
                    # the group loop (skips the separate f32 v_sb + copy)
                    v16 = sp.tile([B, JC], fp16, tag="s_sb")
                    nc.vector.tensor_mul(
                        v16[:].rearrange("p (c j) -> p c j", j=J),
                        v0[:].rearrange("p (c j) -> p c j", j=J),
                        invd[:].rearrange("p (one j) -> p one j", one=1)
                               .broadcast_to((B, C, J)))
                    v_rep = constp.tile([128, JC], fp16, tag="v_rep")
                    for g in range(G):
                        nc.sync.dma_start(v_rep[g * B:(g + 1) * B, :], v16[:])

                    s_ps = spp.tile([B, JC], f32, tag="s")

                    def stage_b(u_t, e_grp, lns, g0):
                        # s += (e/Z) * u: e applied on DVE through a broadcast
                        # view of e_grp (middle-axis stride-0, innermost j
                        # unit-stride -> still 2x); 1/Z rides the stationary.
                        prod2 = wp.tile([128, GR * JC], fp16, tag="prod1", bufs=3)
                        nc.vector.tensor_mul(
                            prod2[:].rearrange("p (n c j) -> p n c j",
                                               n=GR, j=J),
                            u_t[:].rearrange("p (n c j) -> p n c j",
                                             n=GR, j=J),
                            e_grp[:].rearrange("p (n o j) -> p n o j", n=GR, o=1)
                                    .broadcast_to((128, GR, C, J)))
                        for n in range(GR):
                            blk = g0 + n
                            for ch in range(NCH):
                                sl = slice(n * JC + ch * 512,
                                           n * JC + (ch + 1) * 512)
                                nc.tensor.matmul(s_ps[:, ch * 512:(ch + 1) * 512],
                                                 lns[n][:], prod2[:, sl],
                                                 start=(blk == 0),
                                                 stop=(blk == NBLK - 1))

                    pending = None
                    for g0 in range(0, NBLK, GR):
                        b_g = b_tiles[g0 // GR]
                        u_t = up.tile([128, GR * JC], fp16, tag="u_t", bufs=3)
                        nc.sync.dma_start(
                            u_t[:].rearrange("p (n f) -> p n f", n=GR),
                            u_store[g0:g0 + GR].rearrange("n p f -> p n f"))
                        # logits update t = sum_c u*v: DVE mul + pairwise fp16
                        # tree over the outer c axis (all levels contiguous
                        # 64-wide j-runs -> 2x mode).
                        prod1 = wp.tile([128, GR * JC], fp16, tag="prod1", bufs=3)
                        nc.vector.tensor_mul(
                            prod1[:].rearrange("p (n f) -> p n f", n=GR),
                            u_t[:].rearrange("p (n f) -> p n f", n=GR),
                            v_rep[:].rearrange("p (o f) -> p o f", o=1)
                                    .broadcast_to((128, GR, JC)))
                        p4 = prod1[:].rearrange("p (n c j) -> p n c j",
                                                n=GR, j=J)
                        nc.vector.tensor_add(p4[:, :, 0:16, :], p4[:, :, 0:16, :],
                                             p4[:, :, 16:32, :])
                        nc.vector.tensor_add(p4[:, :, 0:8, :], p4[:, :, 0:8, :],
                                             p4[:, :, 8:16, :])
                        nc.vector.tensor_add(p4[:, :, 0:4, :], p4[:, :, 0:4, :],
                                             p4[:, :, 4:8, :])
                        nc.vector.tensor_add(p4[:, :, 0:2, :], p4[:, :, 0:2, :],
                                             p4[:, :, 2:4, :])
                        t16 = spg.tile([128, GR * J], fp16, tag="t16", bufs=2)
                        nc.vector.tensor_add(
                            t16[:].rearrange("p (n o j) -> p n o j", n=GR, o=1),
                            p4[:, :, 0:1, :], p4[:, :, 1:2, :])
                        # b += t on DVE: mixed f32+fp16 is 1x but tiny, and
                        # avoids a cross-engine hop (Pool adds cost ~4us/op)
                        nc.vector.tensor_add(b_g[:], b_g[:], t16[:])
                        # core-local softmax over j (all J present).  Z comes
                        # free from the exp's accum_out; 1/Z is folded into
                        # the PE stationary ln_n = d1 * invz_n, built on ACT
                        # where AP-scalar reads are cheap (213ns vs 1285ns
                        # for the DVE tensor_scalar BYPASS path).
                        nmx = spg.tile([128, GR], f32, tag="nmx")
                        nc.vector.reduce_max(
                            nmx[:], b_g[:].rearrange("p (n j) -> p n j", n=GR),
                            axis=X, negate=True)
                        e_grp = spg.tile([128, GR * J], fp16, tag="e_grp",
                                         bufs=2)
                        zacc = spg.tile([128, GR], f32, tag="zacc")
                        for n in range(GR):
                            nc.scalar.activation(
                                e_grp[:, n * J:(n + 1) * J],
                                b_g[:, n * J:(n + 1) * J],
                                Act.Exp, bias=nmx[:, n:n + 1],
                                accum_out=zacc[:, n:n + 1])
                        invz = spg.tile([128, GR], f32, tag="invz")
                        nc.vector.reciprocal(invz[:], zacc[:])
                        lns = []
                        for n in range(GR):
                            ln = spg.tile([128, B], fp16, tag=f"ln{n}", bufs=2,
                                          name=f"ln{n}")
                            nc.scalar.mul(ln[:], d1_t[:], invz[:, n:n + 1])
                            lns.append(ln)
                        # software pipeline: emit the previous group's
                        # prod2+matmuls after this group's A-stage so DVE/PE
                        # have ready work while ACT runs this group's exp
                        if pending is not None:
                            stage_b(*pending)
                        pending = (u_t, e_grp, lns, g0)
                    stage_b(*pending)

    nc.compile()
    return nc


def _prep_inputs(x, w_ij):
    """Host-side shard + layout. Returns per-core in_maps."""
    x_t = np.ascontiguousarray(x.transpose(1, 2, 0)).astype(np.float16)   # [I,D,B]
    # (c, j) free layout: W arranged [I, D, C, J]
    w_t = np.ascontiguousarray(w_ij.transpose(1, 3, 2, 0)).astype(np.float16)
    d1 = np.tile(np.eye(B, dtype=np.float16), (G, 1))                     # [128,B]
    in_maps = []
    for k in range(NCORES):
        xs = x_t[k * I_LOC:(k + 1) * I_LOC].reshape(NBLK, G, D, B)
        xd = np.zeros((NBLK, G * D, 128), np.float16)
        for g in range(G):
            xd[:, g * D:(g + 1) * D, g * B:(g + 1) * B] = xs[:, g]
        ws = w_t[k * I_LOC:(k + 1) * I_LOC].reshape(NBLK // 2, 2 * G * D, JC)
        # 1/J folded here so iteration 1's s needs no extra scale pass
        xsk = np.ascontiguousarray(
            x_t[k * I_LOC:(k + 1) * I_LOC].reshape(NBLK // 2, 2 * G * D, B)
            .astype(np.float32) / J).astype(np.float16)
        in_maps.append({"xd": xd.reshape(NBLK // 2, 2 * G * D, 128),
                        "xs": xsk, "wm": np.ascontiguousarray(ws),
                        "d1": d1})
    return in_maps


def kernel(x, w_ij, _trace=False):
    from concourse import bass_utils

    if "nc" not in _CACHE:
        _CACHE["nc"] = _build()
    nc = _CACHE["nc"]
    in_maps = _prep_inputs(np.asarray(x), np.asarray(w_ij))
    res = bass_utils.run_bass_kernel_spmd(
        nc, in_maps, core_ids=list(range(NCORES)), trace=_trace)
    _CACHE["last_result"] = res
    # kernel returns [B, (C, J)]; transpose back to [B, J, C] on host
    v = res.results[0]["v_out"].reshape(B, C, J).transpose(0, 2, 1)
    return np.ascontiguousarray(v).astype(np.float32)


# revision 8
# speedup vs baseline: 1.2505x; 1.0242x over previous
"""DenseCapsule dynamic-routing kernel for 8 Trainium2 NeuronCores.

Problem: x[B=32,I=2048,D=16], w_ij[J=64,I=2048,C=32,D=16]
  u_hat = einsum('bid,jicd->bjic', x, w_ij)
  5 routing iterations (softmax over J, s = sum_i c*u_hat, v = squash(s),
  b += sum_c v*u_hat), return v [B,J,C].

Sharding: input capsules I are split 8 ways (I_LOC=256 per core).  The
softmax over J is then core-local; the only collective is an AllReduce of
the per-core partial s [B,J,C] (128 KB fp16) once per iteration.

v3: the free axis of every big tile is (c, j) instead of (j, c).  With j
innermost, the per-(i,j) softmax weight e broadcasts along the MIDDLE
axis (stride-0) while the innermost stays unit-stride fp16 — so DVE's
prod2 reads e_grp directly through a broadcast view in 2x mode and the
[128,8192] e_rep materialization (4 ACT copies, 8us/group, the v0/v2
pipeline stall) disappears.  1/Z is folded into the PE stationary
(ln_n = d1 * invz_n) built on ACT where AP-scalar reads are cheap
(213ns vs 1285ns on DVE).  The c-reduction for the logit update is a
pairwise fp16 tree on DVE (all levels 2x; a single segmented reduce_sum
measures 1x).  Host transposes the [B,(C,J)] result back to [B,J,C].
"""

import numpy as np

B, I, D, J, C = 32, 2048, 16, 64, 32
NCORES = 8
I_LOC = I // NCORES      # 256
G = 4                    # i's per block (G*D = 64 contraction partitions)
NBLK = I_LOC // G        # 64
JC = J * C               # 2048 (free size; layout is (c, j) in-kernel)
ITERS = 5
EPS = 1e-7
NCH = 4                  # 512-wide matmul chunks over the (c,j) free axis
GR = 4                   # i-blocks per phase-2 tile group
NG = NBLK // GR          # 16 groups per iteration

_CACHE = {}


def _build(repeats=1):
    import concourse.bacc as bacc
    import concourse.mybir as mybir
    from concourse import tile

    f32 = mybir.dt.float32
    fp16 = mybir.dt.float16
    Act = mybir.ActivationFunctionType
    Alu = mybir.AluOpType
    X = mybir.AxisListType.X

    nc = bacc.Bacc("TRN2", target_bir_lowering=False, debug=False,
                   num_devices=NCORES)
    NP = NBLK // 2   # block pairs: two G*D=64 blocks stacked on 128 partitions
    xd = nc.dram_tensor("xd", [NP, 128, 128], fp16, kind="ExternalInput").ap()
    xs = nc.dram_tensor("xs", [NP, 128, B], fp16, kind="ExternalInput").ap()
    wm = nc.dram_tensor("wm", [NP, 128, JC], fp16, kind="ExternalInput").ap()
    d1 = nc.dram_tensor("d1", [128, B], fp16, kind="ExternalInput").ap()
    v_out = nc.dram_tensor("v_out", [B, JC], f32, kind="ExternalOutput").ap()

    with tile.TileContext(nc) as tc:
        with tc.tile_pool(name="const", bufs=1) as constp, \
             tc.tile_pool(name="io", bufs=2) as iop, \
             tc.tile_pool(name="u", bufs=1) as up, \
             tc.tile_pool(name="work", bufs=2) as wp, \
             tc.tile_pool(name="small", bufs=1) as sp, \
             tc.tile_pool(name="spg", bufs=3) as spg, \
             tc.tile_pool(name="psum", bufs=4, space="PSUM") as pp, \
             tc.tile_pool(name="spsum", bufs=1, space="PSUM") as spp, \
             tc.tile_pool(name="ud", bufs=1, space="DRAM") as udp, \
             tc.tile_pool(name="ar", bufs=2, space="DRAM") as arp:

            d1_t = constp.tile([128, B], fp16)
            nc.sync.dma_start(d1_t[:], d1[:])
            eps_t = constp.tile([B, 1], f32, tag="eps")
            nc.gpsimd.memset(eps_t[:], EPS)
            b_tiles = []                                 # routing logits
            for gi in range(NG):
                bt = constp.tile([128, GR * J], f32, tag=f"b{gi}")
                b_tiles.append(bt)
            u_store = udp.tile([NBLK, 128, JC], fp16)

            for _rep in range(repeats):
                for bt in b_tiles:
                    nc.gpsimd.memset(bt[:], 0.0)

                # ---- Phase 1: u_hat production + iteration-1 s accumulation
                # s^1 = sum_i u is computed straight from x,W (stationary xs =
                # stacked x block, no block-diagonal) so the s-matmuls depend
                # only on wm_t, never on the PSUM->SBUF copies: PE streams
                # LDW/MM back-to-back with zero DVE/ACT dependencies.
                s_ps = spp.tile([B, JC], f32, tag="s")
                for pr in range(NP):
                    xd_t = iop.tile([128, 128], fp16, tag="xd_t")
                    nc.sync.dma_start(xd_t[:], xd[pr])
                    xs_t = iop.tile([128, B], fp16, tag="xs_t")
                    nc.sync.dma_start(xs_t[:], xs[pr])
                    wm_t = iop.tile([128, JC], fp16, tag="wm_t", bufs=3)
                    nc.sync.dma_start(wm_t[:, :JC // 2], wm[pr][:, :JC // 2])
                    nc.sync.dma_start(wm_t[:, JC // 2:], wm[pr][:, JC // 2:])
                    u16a = iop.tile([128, JC], fp16, tag="u16a")
                    u16b = iop.tile([128, JC], fp16, tag="u16b")
                    for ch in range(NCH):
                        sl = slice(ch * 512, (ch + 1) * 512)
                        psa = pp.tile([128, 512], f32, tag="psa", bufs=2)
                        nc.tensor.matmul(psa[:], xd_t[0:64, :], wm_t[0:64, sl],
                                         start=True, stop=True,
                                         tile_position=(0, 0))
                        psb = pp.tile([128, 512], f32, tag="psb", bufs=2)
                        nc.tensor.matmul(psb[:], xd_t[64:128, :], wm_t[64:128, sl],
                                         start=True, stop=True,
                                         tile_position=(64, 0))
                        # PSUM evac split evenly: DVE cast ~0.62us, ACT copy
                        # ~0.5us per [128,512] tile
                        if ch % 2 == 0:
                            nc.vector.tensor_copy(u16a[:, sl], psa[:])
                            nc.scalar.copy(u16b[:, sl], psb[:])
                        else:
                            nc.scalar.copy(u16a[:, sl], psa[:])
                            nc.vector.tensor_copy(u16b[:, sl], psb[:])
                    for ch in range(NCH):
                        sl = slice(ch * 512, (ch + 1) * 512)
                        nc.tensor.matmul(s_ps[:, sl], xs_t[:], wm_t[:, sl],
                                         start=(pr == 0), stop=(pr == NP - 1))
                    nc.sync.dma_start(u_store[2 * pr], u16a[:])
                    nc.sync.dma_start(u_store[2 * pr + 1], u16b[:])

                # ---- Phase 2: routing iterations
                for it in range(1, ITERS + 1):
                    # v^{it} from the s accumulated for iteration `it`.
                    # AllReduce payload in fp16 (halves collective bytes; the
                    # 8-way partial sums tolerate fp16 rounding).
                    s_sb = sp.tile([B, JC], fp16, tag="s_sb")
                    nc.scalar.activation(s_sb[:], s_ps[:], Act.Copy, bias=0.0)
                    ar_in = arp.tile([B, JC], fp16, tag="ar_in")
                    ar_out = arp.tile([B, JC], fp16, tag="ar_out")
                    nc.sync.dma_start(ar_in[:], s_sb[:])
                    nc.gpsimd.collective_compute(
                        "AllReduce", Alu.add,
                        replica_groups=[list(range(NCORES))],
                        ins=[ar_in.opt()], outs=[ar_out.opt()],
                    )
                    s_full = sp.tile([B, JC], fp16, tag="s_full")
                    nc.sync.dma_start(s_full[:], ar_out[:])

                    # squash: v = (s+eps) * scale, scale = sqrt(n)/(1+n),
                    # n = sum_c (s+eps)^2.  sq on ACT (fused +EPS bias); the
                    # c-reduction is a contiguous fp16 halving tree (2x mode,
                    # c is the outer free axis).
                    sq = sp.tile([B, JC], fp16, tag="s_sb")
                    nc.scalar.activation(sq[:], s_full[:], Act.Square,
                                         bias=eps_t[:])
                    h = JC // 2
                    while h >= J:
                        nc.vector.tensor_add(sq[:, 0:h], sq[:, 0:h],
                                             sq[:, h:2 * h])
                        h //= 2
                    norm = sp.tile([B, J], f32, tag="norm")
                    nc.vector.tensor_copy(norm[:], sq[:, 0:J])
                    rt = sp.tile([B, J], f32, tag="rt")
                    nc.scalar.activation(rt[:], norm[:], Act.Sqrt)
                    np1 = sp.tile([B, J], f32, tag="np1")
                    nc.vector.tensor_scalar_add(np1[:], norm[:], 1.0)
                    inv1 = sp.tile([B, J], f32, tag="inv1")
                    nc.vector.reciprocal(inv1[:], np1[:])
                    if it == ITERS:
                        invd = sp.tile([B, J], f32, tag="invd")
                        nc.vector.tensor_mul(invd[:], rt[:], inv1[:])
                        v_sb = sp.tile([B, JC], f32, tag="v0")
                        nc.vector.scalar_tensor_tensor(
                            v_sb[:].rearrange("p (c j) -> p c j", j=J),
                            s_full[:].rearrange("p (c j) -> p c j", j=J),
                            EPS,
                            invd[:].rearrange("p (one j) -> p one j", one=1)
                                   .broadcast_to((B, C, J)),
                            op0=Alu.add, op1=Alu.mult)
                        nc.sync.dma_start(v_out[:], v_sb[:])
                        break

                    # v16 = (s+eps)*invd fused in one DVE pass; fp16 invd
                    # keeps every operand 2B so the STT runs in 2x mode
                    invd = sp.tile([B, J], fp16, tag="invd")
                    nc.vector.tensor_mul(invd[:], rt[:], inv1[:])
                    v16 = sp.tile([B, JC], fp16, tag="s_sb")
                    nc.vector.scalar_tensor_tensor(
                        v16[:].rearrange("p (c j) -> p c j", j=J),
                        s_full[:].rearrange("p (c j) -> p c j", j=J),
                        EPS,
                        invd[:].rearrange("p (one j) -> p one j", one=1)
                               .broadcast_to((B, C, J)),
                        op0=Alu.add, op1=Alu.mult)
                    v_rep = constp.tile([128, JC], fp16, tag="v_rep")
                    for g in range(G):
                        nc.sync.dma_start(v_rep[g * B:(g + 1) * B, :], v16[:])

                    s_ps = spp.tile([B, JC], f32, tag="s")

                    def stage_b(u_t, e_grp, lns, g0):
                        # s += (e/Z) * u: e applied on DVE through a broadcast
                        # view of e_grp (middle-axis stride-0, innermost j
                        # unit-stride -> still 2x); 1/Z rides the stationary.
                        prod2 = wp.tile([128, GR * JC], fp16, tag="prod1", bufs=3)
                        nc.vector.tensor_mul(
                            prod2[:].rearrange("p (n c j) -> p n c j",
                                               n=GR, j=J),
                            u_t[:].rearrange("p (n c j) -> p n c j",
                                             n=GR, j=J),
                            e_grp[:].rearrange("p (n o j) -> p n o j", n=GR, o=1)
                                    .broadcast_to((128, GR, C, J)))
                        for n in range(GR):
                            blk = g0 + n
                            for ch in range(NCH):
                                sl = slice(n * JC + ch * 512,
                                           n * JC + (ch + 1) * 512)
                                nc.tensor.matmul(s_ps[:, ch * 512:(ch + 1) * 512],
                                                 lns[n][:], prod2[:, sl],
                                                 start=(blk == 0),
                                                 stop=(blk == NBLK - 1))

                    pending = None
                    for g0 in range(0, NBLK, GR):
                        b_g = b_tiles[g0 // GR]
                        u_t = up.tile([128, GR * JC], fp16, tag="u_t", bufs=3)
                        nc.sync.dma_start(
                            u_t[:].rearrange("p (n f) -> p n f", n=GR),
                            u_store[g0:g0 + GR].rearrange("n p f -> p n f"))
                        # logits update t = sum_c u*v: DVE mul + pairwise fp16
                        # tree over the outer c axis (all levels contiguous
                        # 64-wide j-runs -> 2x mode).
                        prod1 = wp.tile([128, GR * JC], fp16, tag="prod1", bufs=3)
                        nc.vector.tensor_mul(
                            prod1[:].rearrange("p (n f) -> p n f", n=GR),
                            u_t[:].rearrange("p (n f) -> p n f", n=GR),
                            v_rep[:].rearrange("p (o f) -> p o f", o=1)
                                    .broadcast_to((128, GR, JC)))
                        p4 = prod1[:].rearrange("p (n c j) -> p n c j",
                                                n=GR, j=J)
                        nc.vector.tensor_add(p4[:, :, 0:16, :], p4[:, :, 0:16, :],
                                             p4[:, :, 16:32, :])
                        nc.vector.tensor_add(p4[:, :, 0:8, :], p4[:, :, 0:8, :],
                                             p4[:, :, 8:16, :])
                        nc.vector.tensor_add(p4[:, :, 0:4, :], p4[:, :, 0:4, :],
                                             p4[:, :, 4:8, :])
                        nc.vector.tensor_add(p4[:, :, 0:2, :], p4[:, :, 0:2, :],
                                             p4[:, :, 2:4, :])
                        t16 = spg.tile([128, GR * J], fp16, tag="t16", bufs=2)
                        nc.vector.tensor_add(
                            t16[:].rearrange("p (n o j) -> p n o j", n=GR, o=1),
                            p4[:, :, 0:1, :], p4[:, :, 1:2, :])
                        # b += t on DVE: mixed f32+fp16 is 1x but tiny, and
                        # avoids a cross-engine hop (Pool adds cost ~4us/op)
                        nc.vector.tensor_add(b_g[:], b_g[:], t16[:])
                        # core-local softmax over j (all J present).  Z comes
                        # free from the exp's accum_out; 1/Z is folded into
                        # the PE stationary ln_n = d1 * invz_n, built on ACT
                        # where AP-scalar reads are cheap (213ns vs 1285ns
                        # for the DVE tensor_scalar BYPASS path).
                        nmx = spg.tile([128, GR], f32, tag="nmx")
                        nc.vector.reduce_max(
                            nmx[:], b_g[:].rearrange("p (n j) -> p n j", n=GR),
                            axis=X, negate=True)
                        e_grp = spg.tile([128, GR * J], fp16, tag="e_grp",
                                         bufs=2)
                        zacc = spg.tile([128, GR], f32, tag="zacc")
                        for n in range(GR):
                            nc.scalar.activation(
                                e_grp[:, n * J:(n + 1) * J],
                                b_g[:, n * J:(n + 1) * J],
                                Act.Exp, bias=nmx[:, n:n + 1],
                                accum_out=zacc[:, n:n + 1])
                        invz = spg.tile([128, GR], f32, tag="invz")
                        nc.vector.reciprocal(invz[:], zacc[:])
                        lns = []
                        for n in range(GR):
                            ln = spg.tile([128, B], fp16, tag=f"ln{n}", bufs=2,
                                          name=f"ln{n}")
                            nc.scalar.mul(ln[:], d1_t[:], invz[:, n:n + 1])
                            lns.append(ln)
                        # software pipeline: emit the previous group's
                        # prod2+matmuls after this group's A-stage so DVE/PE
                        # have ready work while ACT runs this group's exp
                        if pending is not None:
                            stage_b(*pending)
                        pending = (u_t, e_grp, lns, g0)
                    stage_b(*pending)

    nc.compile()
    return nc


def _prep_inputs(x, w_ij):
    """Host-side shard + layout. Returns per-core in_maps."""
    x_t = np.ascontiguousarray(x.transpose(1, 2, 0)).astype(np.float16)   # [I,D,B]
    # (c, j) free layout: W arranged [I, D, C, J]
    w_t = np.ascontiguousarray(w_ij.transpose(1, 3, 2, 0)).astype(np.float16)
    d1 = np.tile(np.eye(B, dtype=np.float16), (G, 1))                     # [128,B]
    in_maps = []
    for k in range(NCORES):
        xs = x_t[k * I_LOC:(k + 1) * I_LOC].reshape(NBLK, G, D, B)
        xd = np.zeros((NBLK, G * D, 128), np.float16)
        for g in range(G):
            xd[:, g * D:(g + 1) * D, g * B:(g + 1) * B] = xs[:, g]
        ws = w_t[k * I_LOC:(k + 1) * I_LOC].reshape(NBLK // 2, 2 * G * D, JC)
        # 1/J folded here so iteration 1's s needs no extra scale pass
        xsk = np.ascontiguousarray(
            x_t[k * I_LOC:(k + 1) * I_LOC].reshape(NBLK // 2, 2 * G * D, B)
            .astype(np.float32) / J).astype(np.float16)
        in_maps.append({"xd": xd.reshape(NBLK // 2, 2 * G * D, 128),
                        "xs": xsk, "wm": np.ascontiguousarray(ws),
                        "d1": d1})
    return in_maps


def kernel(x, w_ij, _trace=False):
    from concourse import bass_utils

    if "nc" not in _CACHE:
        _CACHE["nc"] = _build()
    nc = _CACHE["nc"]
    in_maps = _prep_inputs(np.asarray(x), np.asarray(w_ij))
    res = bass_utils.run_bass_kernel_spmd(
        nc, in_maps, core_ids=list(range(NCORES)), trace=_trace)
    _CACHE["last_result"] = res
    # kernel returns [B, (C, J)]; transpose back to [B, J, C] on host
    v = res.results[0]["v_out"].reshape(B, C, J).transpose(0, 2, 1)
    return np.ascontiguousarray(v).astype(np.float32)
